# revision 43
# baseline (speedup 1.0000x reference)
"""Trainium2 Bass kernel for nn_BinaryClassifier (FFT-frame-mean + 3-layer MLP).

Math: the reference computes sigmoid(relu(relu(Re(mean_f FFT(x_f)) @ W1.T +
b1) @ W2.T + b2) @ W3.T + b3). The frame-mean and the FFT are linear and only
the real part survives, so
    Re(mean_f FFT(x_f)) = (sum_f x_f) @ (C / 31),  C[n,k] = cos(2*pi*n*k/N)
and layer 1 folds to  relu( (sum_f x_f) @ W1c + b1 )  with W1c = C @ W1.T / 31
precomputed on host in float64. Device work = the 31-frame sum (memory bound)
plus a tiny MLP.

v2 over the 123us v1 (which streamed x fp32 and transposed on PE):
- x is shipped fp16 (host cast): halves the HBM stream 32.5 -> 15.9 MB/core.
  The measured per-core DMA rate is ~425 GB/s, so the stream floor drops
  ~80us -> ~40us. fp16 keeps 2^-11 relative error; whole-pipeline numpy
  emulation gives 4.8e-4 max rel err (vs 1.6e-3 for v1's bf16 W1c).
- x is also shipped block-transposed (host layout [n, f, k, b], i.e. feature-
  within-chunk on partitions): the frame-sum lands directly in the [feat,
  batch] layout layer 1 needs, deleting v1's 16 PE transposes + PSUM->SBUF
  bounces + f32r machinery from the post-stream tail.
- All device matmuls are fp16 single-pass (identity frame-sum, W1c, W2, W3);
  DVE adds are fp16 (2-byte dtypes enable the fast DVE modes).
- Frame sum: DVE accumulates 3 fp16 chains; PE identity-matmuls the other 10
  frames into a PSUM fp32 master. The first two chains are folded into PSUM
  by PE mid-stream (hides the merge + keeps fp16 rounding chains short); only
  the last 5-frame chain merges in the tail.
- Tail is quarter-pipelined: f30's add is quartered, each merge quarter
  releases 8 layer-1 matmuls (m0/m1 interleaved).
- The 1.1 MB W1c/W2/W3 fp16 pack is DMA'd mid-stream so the x stream ramps
  immediately; only a 2.5KB bias pack and the 32KB fp16 identity go first.

Sharding: pure data parallel; 1024 batch rows / 8 cores = 128 rows = one SBUF
partition dim per core. Weights replicated.
"""

import os
from contextlib import ExitStack

import numpy as np

import concourse.bacc as bacc
import concourse.bass as bass
import concourse.tile as tile
from concourse import mybir
from concourse.bass_utils import run_bass_kernel_spmd

FRAMES = 31
FFT_LEN = 2048
B = 1024
NCORES = 8
BS = B // NCORES  # 128
H1 = 256
H2 = 256
P = 128
KCH = FFT_LEN // P  # 16 feature chunks

F32 = mybir.dt.float32
F16 = mybir.dt.float16

# fp16 weight pack wh [128, NH] column layout
ID0 = 0  # identity [128]
W1C0 = ID0 + P  # 16 chunks x 256
W2T0 = W1C0 + KCH * H1  # 2 m x 2 j x 128
W3T0 = W2T0 + 2 * H2  # 2 cols
NH = W3T0 + 2
# fp32 bias pack wq [128, NQ]
B10 = 0  # 2 cols
B20 = 2  # 2 cols
B30 = 4  # 1 col
NQ = 5

# frame ownership: PE identity-matmuls these into the PSUM master (each
# costs ~3.6us under HAM k=4 throttle, so PE gets few frames, none near the
# stream end); DVE sums the rest in four short fp16 chains. Chains 1-3 are
# folded into PSUM by PE mid-stream; only chain 4 merges in the tail.
PE_FRAMES = (3, 7, 11, 15, 19)
CHAINS = (
    (0, 1, 2, 4, 5, 6),
    (8, 9, 10, 12, 13, 14),
    (16, 17, 18, 20, 21, 22, 23),
    (24, 25, 26, 27, 28, 29, 30),
)
# PE fold of chain i is EMITTED right after the chain's last frame in the
# loop (program order must place the fold after every add it consumes; the
# PE executes it later, gated on the chain's final DVE add). The last fold
# lands late enough that fold -> ACT copies -> L1 pass 1 finishes right when
# the DVE side (chain 4 + f30 quarters) does -- both sides balanced.
FOLD_EMIT = {6: 0, 14: 1, 23: 2}
# Every frame is its own DMA, alternating rings (even->scalar, odd->sync,
# f30 on sync to balance bytes 16:15): per-queue rates fluctuate +-40% and
# larger groups bunch arrivals AND slow the per-lane in-flight turnover that
# paces DMA issues. Singles arrive every ~1.2us -- the same pace as one DVE
# add -- so the chain tracks the stream with no backlog, and f29/f30 land
# last, in order. ALL weights ride the gpsimd SWDGE queue (a third DMA path
# sharing the same 16 SDMA engines) so the HW rings are pure x streams --
# on-ring weight chunks got hoisted ahead of x by the scheduler and delayed
# every later group on that ring.
GROUPS = tuple((f, 1) for f in range(FRAMES))
TAIL_QUARTERED = (29, 30)  # last two adds quartered + interleaved


def build_nc() -> bass.Bass:
    nc = bacc.Bacc("TRN2", debug=False)

    x_h = nc.dram_tensor("x", [P, FRAMES * FFT_LEN], F16, kind="ExternalInput")
    wq_h = nc.dram_tensor("wq", [P, NQ], F32, kind="ExternalInput")
    wh_h = nc.dram_tensor("wh", [P, NH], F16, kind="ExternalInput")
    out_h = nc.dram_tensor("out", [1, BS], F32, kind="ExternalOutput")

    x3 = x_h.ap().rearrange("p (f n) -> p f n", f=FRAMES)  # [128, 31, 2048]

    with ExitStack() as ctx:
        tc = ctx.enter_context(tile.TileContext(nc))
        singles = ctx.enter_context(tc.tile_pool(name="singles", bufs=1))
        state = ctx.enter_context(tc.tile_pool(name="state", bufs=1))
        frames_pool = ctx.enter_context(tc.tile_pool(name="frames", bufs=1))
        s_ps = ctx.enter_context(tc.tile_pool(name="s_psum", bufs=1, space="PSUM"))
        pl1 = ctx.enter_context(tc.tile_pool(name="pl1", bufs=1, space="PSUM"))
        pwork = ctx.enter_context(tc.tile_pool(name="pwork", bufs=2, space="PSUM"))

        # all weights via gpsimd SWDGE (otherwise idle); lands by ~15us
        wq = singles.tile([P, NQ], F32)
        nc.gpsimd.dma_start(out=wq, in_=wq_h.ap())
        whi = singles.tile([P, P], F16)  # identity
        nc.gpsimd.dma_start(out=whi, in_=wh_h.ap()[:, ID0:P])
        whb = singles.tile([P, NH - P], F16)  # W1c + W2 + W3
        nc.gpsimd.dma_start(out=whb, in_=wh_h.ap()[:, P:NH])

        def w1c(k, m):
            c0 = (W1C0 - P) + k * H1 + m * P
            return whb[:, c0 : c0 + P]

        def w2t(m, j):
            c0 = (W2T0 - P) + m * H2 + j * P
            return whb[:, c0 : c0 + P]

        def w3c(j):
            c0 = (W3T0 - P) + j
            return whb[:, c0 : c0 + 1]

        # pre-joins: let PE/ACT observe the early weight DMAs once so the
        # hot-path instructions keep a single hardware wait slot.
        dummy_ps = pwork.tile([1, 1], F32, tag="pw")
        nc.tensor.matmul(
            dummy_ps, lhsT=whi[:, 0:1], rhs=whi[:, 0:1], start=True, stop=True
        )
        scr = state.tile([1, 1], F32, tag="scr")

        # ---- frame sum into s_psum (PE, fp32) + fp16 DVE chains ----
        s_psum = s_ps.tile([P, FFT_LEN], F32)  # 4 PSUM banks, master accum
        chain_acc = [
            state.tile([P, FFT_LEN], F16, tag=f"s_c{i}", name=f"s_c{i}")
            for i in range(len(CHAINS))
        ]
        psum16 = state.tile([P, FFT_LEN], F16, tag="psum16")  # ACT copy of PSUM

        n_pe = 0  # pe_accum calls done (frames + folds)
        last_pe = len(PE_FRAMES) + len(FOLD_EMIT)  # stop on the final call

        def pe_accum(src, cols=512):
            # start/stop apply to every 512-col chunk of the first/last call:
            # each chunk is a separate PSUM bank whose accumulator must reset
            # on its own first write
            nonlocal n_pe
            for c in range(FFT_LEN // cols):
                nc.tensor.matmul(
                    s_psum[:, c * cols : (c + 1) * cols],
                    lhsT=whi,
                    rhs=src[:, c * cols : (c + 1) * cols],
                    start=(n_pe == 0),
                    stop=(n_pe == last_pe - 1),
                    skip_group_check=True,
                )
            n_pe += 1

        chain_for = {}
        for ci, ch in enumerate(CHAINS):
            for f in ch:
                chain_for[f] = (chain_acc[ci], f == ch[0])

        tail_sl = {}
        for g, (f0, nf) in enumerate(GROUPS):
            xg = frames_pool.tile([P, nf * FFT_LEN], F16, tag="xg", bufs=31)
            eng = nc.sync if (f0 % 2 == 1 or f0 == FRAMES - 1) else nc.scalar
            eng.dma_start(out=xg, in_=x3[:, f0 : f0 + nf, :])
            for j in range(nf):
                f = f0 + j
                sl = xg[:, j * FFT_LEN : (j + 1) * FFT_LEN]
                if f in TAIL_QUARTERED:
                    tail_sl[f] = sl
                elif f in PE_FRAMES:
                    pe_accum(sl)
                else:
                    acc, first = chain_for[f]
                    if first:
                        nc.vector.tensor_copy(acc, sl)
                    else:
                        nc.vector.tensor_add(acc, acc, sl)
                if f in FOLD_EMIT:
                    pe_accum(chain_acc[FOLD_EMIT[f]])

        # ACT pre-joins AFTER the DMA-issue loop: the scalar ENGINE sequences
        # the scalar-ring DMAs, so an early ACT op waiting on the wq DMA
        # would head-of-line block every later x-DMA issue on that ring.
        # Here they run once the issues are all queued, long before the
        # first real ACT op. The Sigmoid also preloads the table (~2.7us
        # cold) during streaming.
        nc.scalar.activation(
            scr, wq[0:1, 0:1], mybir.ActivationFunctionType.Copy, bias=0.0, scale=1.0
        )
        nc.scalar.activation(
            scr, wq[0:1, 0:1], mybir.ActivationFunctionType.Sigmoid, bias=0.0, scale=1.0
        )

        # last two adds quartered + interleaved: each quarter of the final
        # sum is released (and its layer-1 matmuls start) ~1us earlier than
        # full-width adds would allow
        acc = chain_acc[-1]
        Q4 = FFT_LEN // 4
        for q in range(4):
            qs = slice(q * Q4, (q + 1) * Q4)
            for f in TAIL_QUARTERED:
                nc.vector.tensor_add(acc[:, qs], acc[:, qs], tail_sl[f][:, qs])

        # ---- tail: no DVE merge op. h1 = W1c.T @ (psum + c4) is computed as
        # two accumulation passes: pass 1 over psum16 (the ACT fp16 copy of
        # the final PSUM -- runs mid-stream on otherwise-idle ACT/PE), pass 2
        # over the last chain, trailing f30's quartered adds. ----
        Q = FFT_LEN // 4
        for q in range(4):
            qs = slice(q * Q, (q + 1) * Q)
            nc.scalar.activation(
                psum16[:, qs],
                s_psum[:, qs],
                mybir.ActivationFunctionType.Copy,
                bias=0.0,
                scale=1.0,
            )
        h1p = [
            pl1.tile([P, P], F32, tag=f"h1p{m}", name=f"h1p{m}") for m in range(2)
        ]
        for rhs_src, is_last in ((psum16, False), (chain_acc[-1], True)):
            for q in range(4):
                for k in range(q * 4, q * 4 + 4):
                    for m in range(2):
                        nc.tensor.matmul(
                            h1p[m],
                            lhsT=w1c(k, m),
                            rhs=rhs_src[:, k * P : (k + 1) * P],
                            start=(rhs_src is psum16 and k == 0),
                            stop=(is_last and k == KCH - 1),
                            skip_group_check=True,
                        )

        h1_sb = state.tile([P, H1], F16, tag="h1_sb")
        for m in range(2):
            nc.scalar.activation(
                h1_sb[:, m * P : (m + 1) * P],
                h1p[m],
                mybir.ActivationFunctionType.Relu,
                bias=wq[:, B10 + m : B10 + m + 1],
                scale=1.0,
            )

        # ---- layer 2 ----
        h2_sb = state.tile([P, H2], F16, tag="h2_sb")
        for j in range(2):
            h2p = pwork.tile([P, P], F32, tag="pw")
            for m in range(2):
                nc.tensor.matmul(
                    h2p,
                    lhsT=w2t(m, j),
                    rhs=h1_sb[:, m * P : (m + 1) * P],
                    start=(m == 0),
                    stop=(m == 1),
                )
            nc.scalar.activation(
                h2_sb[:, j * P : (j + 1) * P],
                h2p,
                mybir.ActivationFunctionType.Relu,
                bias=wq[:, B20 + j : B20 + j + 1],
                scale=1.0,
            )

        # ---- layer 3 + sigmoid ----
        op = pwork.tile([1, P], F32, tag="pw")
        for j in range(2):
            nc.tensor.matmul(
                op,
                lhsT=w3c(j),
                rhs=h2_sb[:, j * P : (j + 1) * P],
                start=(j == 0),
                stop=(j == 1),
            )
        o_sb = state.tile([1, BS], F32, tag="o_sb")
        nc.scalar.activation(
            o_sb,
            op,
            mybir.ActivationFunctionType.Sigmoid,
            bias=wq[0:1, B30 : B30 + 1],
            scale=1.0,
        )
        # HWDGE out (sync ring is idle by now); avoids the ~1.7us gpsimd
        # SWDGE drain on the critical path
        nc.sync.dma_start(out=out_h.ap(), in_=o_sb)

    nc.compile()
    return nc


_NC_CACHE: dict = {}


def _get_nc() -> bass.Bass:
    if "nc" not in _NC_CACHE:
        _NC_CACHE["nc"] = build_nc()
    return _NC_CACHE["nc"]


_HOST_CACHE: dict = {}


def _host_weights(W1, b1, W2, b2, W3, b3):
    key = (W1.__array_interface__["data"][0], W1.shape)
    if key in _HOST_CACHE:
        return _HOST_CACHE[key]

    n = np.arange(FFT_LEN)
    ang = (2.0 * np.pi / FFT_LEN) * ((n[:, None] * n[None, :]) % FFT_LEN)
    C = np.cos(ang)  # float64 [2048, 2048]
    W1c = (C @ W1.astype(np.float64).T / FRAMES).astype(np.float16)  # [2048, 256]
    W2h = W2.astype(np.float16)  # [256, 256]
    W3h = W3.astype(np.float16).reshape(H2)

    wh = np.zeros((P, NH), dtype=np.float16)
    wh[:, ID0 : ID0 + P] = np.eye(P, dtype=np.float16)
    for k in range(KCH):
        wh[:, W1C0 + k * H1 : W1C0 + (k + 1) * H1] = W1c[k * P : (k + 1) * P, :]
    for m in range(2):
        for j in range(2):
            # lhsT block [o1, o2] = W2[j*128+o2, m*128+o1]
            wh[:, W2T0 + m * H2 + j * P : W2T0 + m * H2 + (j + 1) * P] = W2h[
                j * P : (j + 1) * P, m * P : (m + 1) * P
            ].T
    for j in range(2):
        wh[:, W3T0 + j] = W3h[j * P : (j + 1) * P]

    wq = np.zeros((P, NQ), dtype=np.float32)
    for m in range(2):
        wq[:, B10 + m] = b1.astype(np.float32)[m * P : (m + 1) * P]
        wq[:, B20 + m] = b2.astype(np.float32)[m * P : (m + 1) * P]
    wq[:, B30] = np.float32(b3.reshape(-1)[0])

    pack = {"wq": wq, "wh": wh}
    _HOST_CACHE[key] = pack
    return pack


def kernel(x, W1, b1, W2, b2, W3, b3, _trace=False):
    x = np.asarray(x)
    pack = _host_weights(
        np.asarray(W1), np.asarray(b1), np.asarray(W2),
        np.asarray(b2), np.asarray(W3), np.asarray(b3),
    )
    # fp16 + block-transpose: xh[n, f*2048 + k*128 + b] = x[b, f*2048 + k*128 + n]
    x16 = x.astype(np.float16).reshape(B, FRAMES, KCH, P)
    in_maps = []
    for c in range(NCORES):
        xc = x16[c * BS : (c + 1) * BS]  # [b, f, k, n]
        xh = np.ascontiguousarray(xc.transpose(3, 1, 2, 0)).reshape(P, -1)
        in_maps.append({"x": xh, **pack})
    nc = _get_nc()
    res = run_bass_kernel_spmd(
        nc, in_maps, core_ids=list(range(NCORES)), trace=_trace
    )
    out = np.concatenate([res.results[c]["out"][0] for c in range(NCORES)])
    out = out.reshape(B, 1).astype(np.float32)
    if _trace:
        return out, res
    return out
